# revision 29
# baseline (speedup 1.0000x reference)
"""Multi-head attention (B=2, S=2048, D=2048, H=16, DH=128, RoPE, non-causal)
on 8 Trainium2 NeuronCores.

Sharding: 2-way data parallel on batch x 4-way tensor parallel on heads.
Core c handles batch c//4 and heads (c%4)*4 .. (c%4)*4+4.

Compute dtype: bf16 on the TensorEngine (fp32 PSUM accumulation, fp32
softmax normalization), which gets FWL weight loads and halves DMA traffic.

Per-core kernel:
  1. QK projections, head-dim-major:  QT/KT[dh, s]  (+ fused RoPE via
     sign-folded sin table and partition-strided rotate-half DMA)
  2. V projection, seq-major:         V[s, dh*4]
  3. Attention, transpose-free: scoresT[k, q] on PE, exp on ACT, out and
     softmax row-sums accumulated on PE (all-ones matmul does the
     partition-axis reduction), normalize with DVE reciprocal+multiply.
  4. Per-head AllGather (groups of 4) of the attention output, overlapped
     with the next head's attention.
  5. Output projection accumulated incrementally per head-group into SBUF,
     for this core's own query slice (cc_rank-based dynamic slicing).
Host gathers the 8 disjoint [512, 2048] fp32 output shards.
"""
import numpy as np

B, S, D, H = 2, 2048, 2048, 16
DH = 128
HLOC = 4                 # heads per core
NCORES = 8
GROUPS = [[0, 1, 2, 3], [4, 5, 6, 7]]
SCALE = 1.0 / np.sqrt(DH)
KT = D // 128            # 16 contraction tiles over the model dim
SC = S // 512            # 4 chunks of 512 along seq
ST = S // 128            # 16 seq tiles of 128
QROWS = S // 4           # 512 output rows per core

_BUILT = None


def _build():
    import concourse.bass as bass
    import concourse.tile as tile
    from concourse import bacc, mybir

    F32 = mybir.dt.float32
    BF16 = mybir.dt.bfloat16
    EXPF = mybir.ActivationFunctionType.Exp

    nc = bacc.Bacc("TRN2", target_bir_lowering=False, debug=False,
                   num_devices=NCORES)

    xT_d = nc.dram_tensor("xT", [D, S], BF16, kind="ExternalInput").ap()
    wqk_d = nc.dram_tensor("wqkT", [D, 2 * HLOC * DH], BF16,
                           kind="ExternalInput").ap()
    wv_d = nc.dram_tensor("wvT", [D, HLOC * DH], BF16,
                          kind="ExternalInput").ap()
    woT_d = nc.dram_tensor("woT", [D, D], BF16, kind="ExternalInput").ap()
    cosT_d = nc.dram_tensor("cosT", [DH, S], F32, kind="ExternalInput").ap()
    sinT_d = nc.dram_tensor("sinTs", [DH, S], F32, kind="ExternalInput").ap()
    ones_d = nc.dram_tensor("ones", [128, 128], F32,
                            kind="ExternalInput").ap()
    out_d = nc.dram_tensor("out", [QROWS, D], F32, kind="ExternalOutput").ap()

    with tile.TileContext(nc) as tc:
        with (
            tc.tile_pool(name="dram", bufs=1, space="DRAM") as dram,
            tc.tile_pool(name="onesp", bufs=1) as onesp,
        ):
            ones_sb = onesp.tile([128, 128], F32)
            nc.sync.dma_start(ones_sb[:], ones_d[:])

            ag_ins = []
            ag_outs = []
            for h in range(HLOC):
                ag_ins.append(dram.tile([DH, S], BF16, name=f"agin{h}"))
                ag_outs.append(dram.tile([4 * DH, S], BF16, name=f"agout{h}"))

            # tiny dummy AllGather to absorb the first-collective warmup
            # cost under the projection phase
            warm_in = dram.tile([128, 128], BF16, name="warmin")
            warm_out = dram.tile([512, 128], BF16, name="warmout")
            nc.sync.dma_start(warm_in[:], ones_d[0:128, 0:64].bitcast(BF16))
            nc.gpsimd.collective_compute(
                "AllGather", mybir.AluOpType.bypass,
                replica_groups=GROUPS,
                ins=[warm_in.opt()], outs=[warm_out.opt()],
            )

            with (
                tc.tile_pool(name="qkr", bufs=1) as qkrp,
                tc.tile_pool(name="vper", bufs=1) as vperp,
            ):
                # persistent RoPE'd Q/K, head-dim-major: [dh=128, s=2048] bf16
                QTr = [qkrp.tile([DH, S], BF16, name=f"qtr{h}")
                       for h in range(HLOC)]
                KTr = [qkrp.tile([DH, S], BF16, name=f"ktr{h}")
                       for h in range(HLOC)]
                # persistent V, seq-major: 16 tiles [128 s, 512 = 4 heads]
                V_sb = [vperp.tile([128, HLOC * DH], BF16,
                                   name=f"v{st}") for st in range(ST)]

                # ---------------- Phase 1: Q/K projections + RoPE ----------
                with (
                    tc.tile_pool(name="wqk", bufs=1) as wqkp,
                    tc.tile_pool(name="wvp", bufs=1) as wvp,
                    tc.tile_pool(name="cs", bufs=1) as csp,
                    tc.tile_pool(name="xqk", bufs=36) as xqkp,
                    tc.tile_pool(name="ropew", bufs=4) as ropep,
                    tc.tile_pool(name="psqk", bufs=8, space="PSUM") as psqk,
                ):
                    wqk_sb = [wqkp.tile([128, 2 * HLOC * DH], BF16,
                                        name=f"wqk{kt}") for kt in range(KT)]
                    wv_sb = [wvp.tile([128, HLOC * DH], BF16,
                                      name=f"wv{kt}") for kt in range(KT)]
                    cos_sb = csp.tile([DH, S], F32)
                    sin_sb = csp.tile([DH, S], F32)
                    # first tiles the PE needs go first in the DMA queues
                    nc.sync.dma_start(wqk_sb[0][:], wqk_d[0:128, :])
                    nc.sync.dma_start(wv_sb[0][:], wv_d[0:128, :])

                    def _rope_evac(sc, ssl, t, ps):
                        h, isk = t // 2, t % 2
                        dst = (KTr[h] if isk else QTr[h])
                        plain = ropep.tile([128, 512], F32, tag="plain",
                                           name=f"pl{sc}_{t}")
                        nc.scalar.copy(plain[:], ps[:])
                        tmpc = ropep.tile([128, 512], F32, tag="tmpc",
                                          name=f"tc{sc}_{t}")
                        nc.vector.tensor_mul(tmpc[:], ps[:], cos_sb[:, ssl])
                        rot = ropep.tile([128, 512], F32, tag="rot",
                                         name=f"ro{sc}_{t}")
                        nc.scalar.dma_start(rot[0:64, :], plain[1::2, :])
                        nc.scalar.dma_start(rot[64:128, :], plain[0::2, :])
                        rot2 = ropep.tile([128, 512], F32, tag="rot2",
                                          name=f"ro2{sc}_{t}")
                        nc.vector.tensor_mul(rot2[:], rot[:], sin_sb[:, ssl])
                        nc.vector.tensor_add(dst[:, ssl], tmpc[:], rot2[:])

                    for sc in range(SC):
                        ssl = slice(sc * 512, (sc + 1) * 512)
                        xcs = []
                        for kt in range(KT):
                            xc = xqkp.tile([128, 512], BF16, tag="xc",
                                           name=f"xc{sc}_{kt}")
                            nc.sync.dma_start(
                                xc[:], xT_d[kt * 128:(kt + 1) * 128, ssl])
                            xcs.append(xc)
                            if sc == 0 and kt + 1 < KT:
                                nc.sync.dma_start(
                                    wqk_sb[kt + 1][:],
                                    wqk_d[(kt + 1) * 128:(kt + 2) * 128, :])
                                nc.sync.dma_start(
                                    wv_sb[kt + 1][:],
                                    wv_d[(kt + 1) * 128:(kt + 2) * 128, :])
                            if sc == 0 and kt == KT - 1:
                                nc.sync.dma_start(cos_sb[:], cosT_d[:])
                                nc.sync.dma_start(sin_sb[:], sinT_d[:])

                        def _qk_round(ts, sc=sc, ssl=ssl, xcs=xcs):
                            pss = [psqk.tile([128, 512], F32, tag="qkps",
                                             bufs=4, name=f"qkps{sc}_{t}")
                                   for t in ts]
                            for kt in range(KT):
                                for i, t in enumerate(ts):
                                    nc.tensor.matmul(
                                        pss[i][:],
                                        wqk_sb[kt][:, t * 128:(t + 1) * 128],
                                        xcs[kt][:],
                                        start=(kt == 0), stop=(kt == KT - 1))
                            for i, t in enumerate(ts):
                                _rope_evac(sc, ssl, t, pss[i])

                        _qk_round(range(0, 4))
                        # V round: this s-chunk's 4 seq-tiles, reusing the
                        # staged xc tiles as the stationary operand
                        pvs = [psqk.tile([128, 512], F32, tag="vps",
                                         bufs=4, name=f"vps{sc}_{i}")
                               for i in range(4)]
                        for kt in range(KT):
                            for i in range(4):
                                nc.tensor.matmul(
                                    pvs[i][:],
                                    xcs[kt][:, i * 128:(i + 1) * 128],
                                    wv_sb[kt][:],
                                    start=(kt == 0), stop=(kt == KT - 1))
                        for i in range(4):
                            nc.scalar.copy(V_sb[sc * 4 + i][:], pvs[i][:])
                        _qk_round(range(4, 8))

                with tc.tile_pool(name="postproj", bufs=1) as _postp:
                    # -------- Phase 3+4: attention + AllGather + wo --------
                    with (
                        tc.tile_pool(name="wo", bufs=1) as wop,
                        tc.tile_pool(name="woacc", bufs=1) as woaccp,
                        tc.tile_pool(name="outT", bufs=1) as outTp,
                        tc.tile_pool(name="ao", bufs=2) as aop,
                        tc.tile_pool(name="atw", bufs=3) as atw,
                        tc.tile_pool(name="psat", bufs=1, space="PSUM") as psat,
                    ):
                        rank = nc.sync.cc_rank(replica_groups=GROUPS)
                        # dynamic offsets expressed in f32 units (bf16 pairs):
                        # dynamic ds() on a bf16 AP faults the device
                        qoff32 = rank * 256

                        # woT resident in bf16, loaded during attention
                        wo_sb = [wop.tile([128, D], BF16, name=f"wos{g}")
                                 for g in range(16)]
                        for g in range(16):
                            nc.sync.dma_start(
                                wo_sb[g][:], woT_d[g * 128:(g + 1) * 128, :])

                        wo_acc = [woaccp.tile([128, 512], F32,
                                              name=f"wacc{qt}_{dc}")
                                  for qt in range(4) for dc in range(4)]
                        outT_sb = [outTp.tile([DH, S], BF16, name=f"ot{h}")
                                   for h in range(HLOC)]

                        def _wo_load(h):
                            """Fetch this core's query-slice of AG[h]."""
                            ao_h = [aop.tile([128, 512], BF16, tag=f"aoh{j}",
                                             name=f"aoh{h}_{j}")
                                    for j in range(4)]
                            ag32 = ag_outs[h].bitcast(F32)
                            for j in range(4):
                                nc.sync.dma_start(
                                    ao_h[j][:].bitcast(F32),
                                    ag32[j * 128:(j + 1) * 128,
                                         bass.ds(qoff32, 256)])
                            return ao_h

                        def _wo_chunk(h, ao_h):
                            """Accumulate head-group h's wo contribution."""
                            for qt in range(4):
                                for dc in range(4):
                                    psw = psat.tile([128, 512], F32,
                                                    tag="wops", bufs=2,
                                                    name=f"wps{h}_{qt}_{dc}")
                                    for j in range(4):
                                        nc.tensor.matmul(
                                            psw[:],
                                            ao_h[j][:,
                                                    qt * 128:(qt + 1) * 128],
                                            wo_sb[4 * j + h][:,
                                                             dc * 512:
                                                             (dc + 1) * 512],
                                            start=(j == 0), stop=(j == 3))
                                    acc = wo_acc[qt * 4 + dc]
                                    if h == 0:
                                        nc.scalar.copy(acc[:], psw[:])
                                    elif h == HLOC - 1:
                                        nc.vector.tensor_add(acc[:], acc[:],
                                                             psw[:])
                                        nc.sync.dma_start(
                                            out_d[qt * 128:(qt + 1) * 128,
                                                  dc * 512:(dc + 1) * 512],
                                            acc[:])
                                    else:
                                        nc.vector.tensor_add(acc[:], acc[:],
                                                             psw[:])

                        def _flush_round(p):
                            """softmax-sum matmul + normalization for a
                            finished round, deferred so it overlaps the next
                            round's PE work instead of blocking it."""
                            ph, pqsls, psaccs, pocps = p
                            for s in range(2):
                                sump = psat.tile([128, 512], F32,
                                                 tag="wops", bufs=2,
                                                 name=f"aS{ph}_{pqsls[s].start}")
                                nc.tensor.matmul(sump[:], ones_sb[:],
                                                 psaccs[s][:],
                                                 start=True, stop=True)
                                rec = atw.tile([128, 512], F32, tag="rec",
                                               name=f"rc{ph}_{pqsls[s].start}")
                                nc.vector.reciprocal(rec[:], sump[:])
                                nc.vector.tensor_mul(
                                    outT_sb[ph][:, pqsls[s]],
                                    pocps[s][:].bitcast(F32)
                                    if False else pocps[s][:],
                                    rec[:])

                        pending = None
                        for h in range(HLOC):
                            # two interleaved query-chunk streams: while one
                            # stream waits on exp (ACT), the other keeps the
                            # PE busy
                            for qp in range(SC // 2):
                                qcs = (2 * qp, 2 * qp + 1)
                                qsls = [slice(qc * 512, (qc + 1) * 512)
                                        for qc in qcs]
                                outps = [psat.tile([128, 512], F32,
                                                   tag=f"aout{s}", bufs=1,
                                                   name=f"aO{h}_{qcs[s]}")
                                         for s in range(2)]
                                saccs = [atw.tile([128, 512], F32,
                                                  tag=f"sacc{s}", bufs=2,
                                                  name=f"sA{h}_{qcs[s]}")
                                         for s in range(2)]
                                prevs = [None, None]
                                for kp in range(ST // 2):
                                    scps = []
                                    for s in range(2):
                                        scp = psat.tile(
                                            [128, 1024], F32, tag=f"asc{s}",
                                            bufs=1,
                                            name=f"sc{h}_{qcs[s]}_{kp}")
                                        for j in range(2):
                                            k0 = (2 * kp + j) * 128
                                            nc.tensor.matmul(
                                                scp[:, j * 512:(j + 1) * 512],
                                                KTr[h][:, k0:k0 + 128],
                                                QTr[h][:, qsls[s]],
                                                start=True, stop=True)
                                        scps.append(scp)
                                    for s in range(2):
                                        if prevs[s] is not None:
                                            _attn_tail(nc, atw, prevs[s][1],
                                                       prevs[s][0], outps[s],
                                                       saccs[s], V_sb, h,
                                                       prevs[s][0] == 0,
                                                       False, BF16, EXPF, s)
                                        prevs[s] = (kp, scps[s])
                                ocps = []
                                for s in range(2):
                                    _attn_tail(nc, atw, prevs[s][1],
                                               prevs[s][0], outps[s],
                                               saccs[s], V_sb, h,
                                               prevs[s][0] == 0, True,
                                               BF16, EXPF, s)
                                    # eager bf16 PSUM evacuation frees the
                                    # aout slot without waiting for the
                                    # softmax normalization chain
                                    ocp = atw.tile([128, 512], BF16,
                                                   tag=f"ocp{s}", bufs=2,
                                                   name=f"oc{h}_{qcs[s]}")
                                    nc.vector.tensor_copy(ocp[:], outps[s][:])
                                    ocps.append(ocp)
                                if pending is not None:
                                    _flush_round(pending)
                                pending = (h, qsls, saccs, ocps)
                            # AG needs outT complete: flush before its DMA
                            _flush_round(pending)
                            pending = None
                            ao_prev = _wo_load(h - 1) if h > 0 else None
                            nc.sync.dma_start(ag_ins[h][:], outT_sb[h][:])
                            nc.gpsimd.collective_compute(
                                "AllGather", mybir.AluOpType.bypass,
                                replica_groups=GROUPS,
                                ins=[ag_ins[h].opt()],
                                outs=[ag_outs[h].opt()],
                            )
                            if h > 0:
                                _wo_chunk(h - 1, ao_prev)
                        _wo_chunk(HLOC - 1, _wo_load(HLOC - 1))

    nc.compile()
    return nc


def _attn_tail(nc, atw, scp, kp, outp, sacc, V_sb, h, is_first,
               is_last, BF16, EXPF, s=0):
    """exp + out accumulation (PE) + softmax-sum accumulation (DVE)."""
    F32 = sacc.dtype
    ex = atw.tile([128, 1024], BF16, tag=f"exp{s}", name=f"ex{h}_{kp}_{s}")
    nc.scalar.activation(ex[:], scp[:], EXPF, scale=SCALE)
    for j in range(2):
        st = 2 * kp + j
        nc.tensor.matmul(outp[:], V_sb[st][:, h * 128:(h + 1) * 128],
                         ex[:, j * 512:(j + 1) * 512],
                         start=(is_first and j == 0),
                         stop=(is_last and j == 1))
    fold = atw.tile([128, 512], F32, tag=f"fold{s}", bufs=2,
                    name=f"fo{h}_{kp}_{s}")
    nc.vector.tensor_add(fold[:], ex[:, 0:512], ex[:, 512:1024])
    if is_first:
        nc.vector.tensor_copy(sacc[:], fold[:])
    else:
        nc.vector.tensor_add(sacc[:], sacc[:], fold[:])


def _prep_inputs(x, wq, wk, wv, wo, cos, sin):
    """Host-side sharding/layout prep. Returns per-core input dicts."""
    import ml_dtypes
    bf16 = ml_dtypes.bfloat16

    woT = np.ascontiguousarray(wo.T).astype(bf16)         # [E, D]
    cosT = np.ascontiguousarray(cos[:S, :DH].T)           # [DH, S] f32
    sinT = np.ascontiguousarray(sin[:S, :DH].T)
    sinTs = sinT.copy()
    sinTs[:DH // 2] *= -1.0                               # sign-folded rotate
    ones = np.ones((128, 128), np.float32)
    xTs = [np.ascontiguousarray(x[b].T).astype(bf16) for b in range(B)]

    in_maps = []
    for c in range(NCORES):
        hsel = slice((c % 4) * HLOC * DH, ((c % 4) + 1) * HLOC * DH)
        wq_c = wq[hsel, :]                                # [512, D]
        wk_c = wk[hsel, :]
        qk_cols = np.empty((2 * HLOC * DH, D), np.float32)
        for h in range(HLOC):
            qk_cols[(2 * h) * DH:(2 * h + 1) * DH] = \
                wq_c[h * DH:(h + 1) * DH]
            qk_cols[(2 * h + 1) * DH:(2 * h + 2) * DH] = \
                wk_c[h * DH:(h + 1) * DH]
        wqkT = np.ascontiguousarray(qk_cols.T).astype(bf16)   # [D, 1024]
        wvT = np.ascontiguousarray(wv[hsel, :].T).astype(bf16)  # [D, 512]
        in_maps.append({
            "xT": xTs[c // 4],
            "wqkT": wqkT,
            "wvT": wvT,
            "woT": woT,
            "cosT": cosT,
            "sinTs": sinTs,
            "ones": ones,
        })
    return in_maps


def kernel(x, wq, wk, wv, wo, cos, sin):
    global _BUILT
    from concourse.bass_utils import run_bass_kernel_spmd

    if _BUILT is None:
        _BUILT = _build()
    nc = _BUILT

    in_maps = _prep_inputs(
        np.asarray(x, np.float32), np.asarray(wq, np.float32),
        np.asarray(wk, np.float32), np.asarray(wv, np.float32),
        np.asarray(wo, np.float32), np.asarray(cos, np.float32),
        np.asarray(sin, np.float32))

    res = run_bass_kernel_spmd(nc, in_maps, core_ids=list(range(NCORES)))

    out = np.empty((B, S, D), np.float32)
    for c in range(NCORES):
        out[c // 4, (c % 4) * QROWS:((c % 4) + 1) * QROWS, :] = \
            res.results[c]["out"]
    return out


# revision 30
# speedup vs baseline: 1.0454x; 1.0454x over previous
"""Multi-head attention (B=2, S=2048, D=2048, H=16, DH=128, RoPE, non-causal)
on 8 Trainium2 NeuronCores.

Sharding: 2-way data parallel on batch x 4-way tensor parallel on heads.
Core c handles batch c//4 and heads (c%4)*4 .. (c%4)*4+4.

Compute dtype: bf16 on the TensorEngine (fp32 PSUM accumulation, fp32
softmax normalization), which gets FWL weight loads and halves DMA traffic.

Per-core kernel:
  1. QK projections, head-dim-major:  QT/KT[dh, s]  (+ fused RoPE via
     sign-folded sin table and partition-strided rotate-half DMA)
  2. V projection, seq-major:         V[s, dh*4]
  3. Attention, transpose-free: scoresT[k, q] on PE, exp on ACT, out and
     softmax row-sums accumulated on PE (all-ones matmul does the
     partition-axis reduction), normalize with DVE reciprocal+multiply.
  4. Per-head AllGather (groups of 4) of the attention output, overlapped
     with the next head's attention.
  5. Output projection accumulated incrementally per head-group into SBUF,
     for this core's own query slice (cc_rank-based dynamic slicing).
Host gathers the 8 disjoint [512, 2048] fp32 output shards.
"""
import numpy as np

B, S, D, H = 2, 2048, 2048, 16
DH = 128
HLOC = 4                 # heads per core
NCORES = 8
GROUPS = [[0, 1, 2, 3], [4, 5, 6, 7]]
SCALE = 1.0 / np.sqrt(DH)
KT = D // 128            # 16 contraction tiles over the model dim
SC = S // 512            # 4 chunks of 512 along seq
ST = S // 128            # 16 seq tiles of 128
QROWS = S // 4           # 512 output rows per core

_BUILT = None


def _build():
    import concourse.bass as bass
    import concourse.tile as tile
    from concourse import bacc, mybir

    F32 = mybir.dt.float32
    BF16 = mybir.dt.bfloat16
    EXPF = mybir.ActivationFunctionType.Exp

    nc = bacc.Bacc("TRN2", target_bir_lowering=False, debug=False,
                   num_devices=NCORES)

    xT_d = nc.dram_tensor("xT", [D, S], BF16, kind="ExternalInput").ap()
    wqk_d = nc.dram_tensor("wqkT", [D, 2 * HLOC * DH], BF16,
                           kind="ExternalInput").ap()
    wv_d = nc.dram_tensor("wvT", [D, HLOC * DH], BF16,
                          kind="ExternalInput").ap()
    woT_d = nc.dram_tensor("woT", [D, D], BF16, kind="ExternalInput").ap()
    cosT_d = nc.dram_tensor("cosT", [DH, S], F32, kind="ExternalInput").ap()
    sinT_d = nc.dram_tensor("sinTs", [DH, S], F32, kind="ExternalInput").ap()
    ones_d = nc.dram_tensor("ones", [128, 128], F32,
                            kind="ExternalInput").ap()
    out_d = nc.dram_tensor("out", [QROWS, D], F32, kind="ExternalOutput").ap()

    with tile.TileContext(nc) as tc:
        with (
            tc.tile_pool(name="dram", bufs=1, space="DRAM") as dram,
            tc.tile_pool(name="onesp", bufs=1) as onesp,
        ):
            ones_sb = onesp.tile([128, 128], F32)
            nc.sync.dma_start(ones_sb[:], ones_d[:])

            ag_ins = []
            ag_outs = []
            for h in range(HLOC):
                ag_ins.append(dram.tile([DH, S], BF16, name=f"agin{h}"))
                ag_outs.append(dram.tile([4 * DH, S], BF16, name=f"agout{h}"))

            # tiny dummy AllGather to absorb the first-collective warmup
            # cost under the projection phase
            warm_in = dram.tile([128, 128], BF16, name="warmin")
            warm_out = dram.tile([512, 128], BF16, name="warmout")
            nc.sync.dma_start(warm_in[:], ones_d[0:128, 0:64].bitcast(BF16))
            nc.gpsimd.collective_compute(
                "AllGather", mybir.AluOpType.bypass,
                replica_groups=GROUPS,
                ins=[warm_in.opt()], outs=[warm_out.opt()],
            )

            with tc.tile_pool(name="qkr", bufs=1) as qkrp:
                # persistent RoPE'd Q/K, head-dim-major: [dh=128, s=2048] bf16
                QTr = [qkrp.tile([DH, S], BF16, name=f"qtr{h}")
                       for h in range(HLOC)]
                KTr = [qkrp.tile([DH, S], BF16, name=f"ktr{h}")
                       for h in range(HLOC)]

                # ---------------- Phase 1: Q/K projections + RoPE ----------
                with (
                    tc.tile_pool(name="wqk", bufs=1) as wqkp,
                    tc.tile_pool(name="cs", bufs=1) as csp,
                    tc.tile_pool(name="xqk", bufs=6) as xqkp,
                    tc.tile_pool(name="ropew", bufs=4) as ropep,
                    tc.tile_pool(name="psqk", bufs=8, space="PSUM") as psqk,
                ):
                    wqk_sb = [wqkp.tile([128, 2 * HLOC * DH], BF16,
                                        name=f"wqk{kt}") for kt in range(KT)]
                    cos_sb = csp.tile([DH, S], F32)
                    sin_sb = csp.tile([DH, S], F32)
                    # first tiles the PE needs go first in the DMA queues
                    nc.sync.dma_start(wqk_sb[0][:], wqk_d[0:128, :])
                    for sc in range(SC):
                        ssl = slice(sc * 512, (sc + 1) * 512)
                        pss = [psqk.tile([128, 512], F32, tag="qkps",
                                         name=f"qkps{sc}_{t}")
                               for t in range(2 * HLOC)]
                        for kt in range(KT):
                            xc = xqkp.tile([128, 512], BF16, tag="xc",
                                           name=f"xc{sc}_{kt}")
                            nc.sync.dma_start(
                                xc[:], xT_d[kt * 128:(kt + 1) * 128, ssl])
                            if sc == 0 and kt + 1 < KT:
                                nc.sync.dma_start(
                                    wqk_sb[kt + 1][:],
                                    wqk_d[(kt + 1) * 128:(kt + 2) * 128, :])
                            if sc == 0 and kt == KT - 1:
                                nc.sync.dma_start(cos_sb[:], cosT_d[:])
                                nc.sync.dma_start(sin_sb[:], sinT_d[:])
                            for t in range(2 * HLOC):
                                nc.tensor.matmul(
                                    pss[t][:],
                                    wqk_sb[kt][:, t * 128:(t + 1) * 128],
                                    xc[:],
                                    start=(kt == 0), stop=(kt == KT - 1))
                        # RoPE evacuation: qkr = psum*cos + rot(psum)*sin_s
                        # (sc=0 writes are plain copies deferred via cos/sin
                        #  arriving later is fine: deps handle ordering)
                        for t in range(2 * HLOC):
                            h, isk = t // 2, t % 2
                            dst = (KTr[h] if isk else QTr[h])
                            plain = ropep.tile([128, 512], F32, tag="plain",
                                               name=f"pl{sc}_{t}")
                            nc.scalar.copy(plain[:], pss[t][:])
                            tmpc = ropep.tile([128, 512], F32, tag="tmpc",
                                              name=f"tc{sc}_{t}")
                            nc.vector.tensor_mul(tmpc[:], pss[t][:],
                                                 cos_sb[:, ssl])
                            rot = ropep.tile([128, 512], F32, tag="rot",
                                             name=f"ro{sc}_{t}")
                            nc.scalar.dma_start(rot[0:64, :], plain[1::2, :])
                            nc.scalar.dma_start(rot[64:128, :],
                                                plain[0::2, :])
                            rot2 = ropep.tile([128, 512], F32, tag="rot2",
                                              name=f"ro2{sc}_{t}")
                            nc.vector.tensor_mul(rot2[:], rot[:],
                                                 sin_sb[:, ssl])
                            nc.vector.tensor_add(dst[:, ssl], tmpc[:],
                                                 rot2[:])

                with tc.tile_pool(name="vper", bufs=1) as vperp:
                    # persistent V, seq-major: 16 tiles [128 s, 512 = 4 heads]
                    V_sb = [vperp.tile([128, HLOC * DH], BF16,
                                       name=f"v{st}") for st in range(ST)]

                    # ---------------- Phase 2: V projection ----------------
                    with (
                        tc.tile_pool(name="wv", bufs=1) as wvp,
                        tc.tile_pool(name="xv", bufs=5) as xvp,
                        tc.tile_pool(name="psv", bufs=8, space="PSUM") as psv,
                    ):
                        wv_sb = [wvp.tile([128, HLOC * DH], BF16,
                                          name=f"wv{kt}") for kt in range(KT)]
                        nc.sync.dma_start(wv_sb[0][:], wv_d[0:128, :])
                        for half in range(2):
                            pvs = [psv.tile([128, 512], F32, tag="vps",
                                            name=f"vps{half}_{i}")
                                   for i in range(8)]
                            for kt in range(KT):
                                xv = xvp.tile([128, 1024], BF16, tag="xv",
                                              name=f"xv{half}_{kt}")
                                nc.sync.dma_start(
                                    xv[:],
                                    xT_d[kt * 128:(kt + 1) * 128,
                                         half * 1024:(half + 1) * 1024])
                                if half == 0 and kt + 1 < KT:
                                    nc.sync.dma_start(
                                        wv_sb[kt + 1][:],
                                        wv_d[(kt + 1) * 128:(kt + 2) * 128, :])
                                for i in range(8):
                                    nc.tensor.matmul(
                                        pvs[i][:],
                                        xv[:, i * 128:(i + 1) * 128],
                                        wv_sb[kt][:],
                                        start=(kt == 0), stop=(kt == KT - 1))
                            for i in range(8):
                                nc.scalar.copy(V_sb[half * 8 + i][:],
                                               pvs[i][:])

                    # -------- Phase 3+4: attention + AllGather + wo --------
                    with (
                        tc.tile_pool(name="wo", bufs=1) as wop,
                        tc.tile_pool(name="woacc", bufs=1) as woaccp,
                        tc.tile_pool(name="outT", bufs=1) as outTp,
                        tc.tile_pool(name="ao", bufs=2) as aop,
                        tc.tile_pool(name="atw", bufs=3) as atw,
                        tc.tile_pool(name="psat", bufs=1, space="PSUM") as psat,
                    ):
                        rank = nc.sync.cc_rank(replica_groups=GROUPS)
                        # dynamic offsets expressed in f32 units (bf16 pairs):
                        # dynamic ds() on a bf16 AP faults the device
                        qoff32 = rank * 256

                        # woT resident in bf16, loaded during attention
                        wo_sb = [wop.tile([128, D], BF16, name=f"wos{g}")
                                 for g in range(16)]
                        for g in range(16):
                            nc.sync.dma_start(
                                wo_sb[g][:], woT_d[g * 128:(g + 1) * 128, :])

                        wo_acc = [woaccp.tile([128, 512], F32,
                                              name=f"wacc{qt}_{dc}")
                                  for qt in range(4) for dc in range(4)]
                        outT_sb = [outTp.tile([DH, S], BF16, name=f"ot{h}")
                                   for h in range(HLOC)]

                        def _wo_load(h):
                            """Fetch this core's query-slice of AG[h]."""
                            ao_h = [aop.tile([128, 512], BF16, tag=f"aoh{j}",
                                             name=f"aoh{h}_{j}")
                                    for j in range(4)]
                            ag32 = ag_outs[h].bitcast(F32)
                            for j in range(4):
                                nc.sync.dma_start(
                                    ao_h[j][:].bitcast(F32),
                                    ag32[j * 128:(j + 1) * 128,
                                         bass.ds(qoff32, 256)])
                            return ao_h

                        def _wo_chunk(h, ao_h):
                            """Accumulate head-group h's wo contribution."""
                            for qt in range(4):
                                for dc in range(4):
                                    psw = psat.tile([128, 512], F32,
                                                    tag="wops", bufs=2,
                                                    name=f"wps{h}_{qt}_{dc}")
                                    for j in range(4):
                                        nc.tensor.matmul(
                                            psw[:],
                                            ao_h[j][:,
                                                    qt * 128:(qt + 1) * 128],
                                            wo_sb[4 * j + h][:,
                                                             dc * 512:
                                                             (dc + 1) * 512],
                                            start=(j == 0), stop=(j == 3))
                                    acc = wo_acc[qt * 4 + dc]
                                    if h == 0:
                                        nc.scalar.copy(acc[:], psw[:])
                                    elif h == HLOC - 1:
                                        nc.vector.tensor_add(acc[:], acc[:],
                                                             psw[:])
                                        nc.sync.dma_start(
                                            out_d[qt * 128:(qt + 1) * 128,
                                                  dc * 512:(dc + 1) * 512],
                                            acc[:])
                                    else:
                                        nc.vector.tensor_add(acc[:], acc[:],
                                                             psw[:])

                        def _flush_round(p):
                            """softmax-sum matmul + normalization for a
                            finished round, deferred so it overlaps the next
                            round's PE work instead of blocking it."""
                            ph, pqsls, psaccs, pocps = p
                            for s in range(2):
                                sump = psat.tile([128, 512], F32,
                                                 tag="wops", bufs=2,
                                                 name=f"aS{ph}_{pqsls[s].start}")
                                nc.tensor.matmul(sump[:], ones_sb[:],
                                                 psaccs[s][:],
                                                 start=True, stop=True)
                                rec = atw.tile([128, 512], F32, tag="rec",
                                               name=f"rc{ph}_{pqsls[s].start}")
                                nc.vector.reciprocal(rec[:], sump[:])
                                nc.vector.tensor_mul(
                                    outT_sb[ph][:, pqsls[s]],
                                    pocps[s][:].bitcast(F32)
                                    if False else pocps[s][:],
                                    rec[:])

                        pending = None
                        for h in range(HLOC):
                            # two interleaved query-chunk streams: while one
                            # stream waits on exp (ACT), the other keeps the
                            # PE busy
                            for qp in range(SC // 2):
                                qcs = (2 * qp, 2 * qp + 1)
                                qsls = [slice(qc * 512, (qc + 1) * 512)
                                        for qc in qcs]
                                outps = [psat.tile([128, 512], F32,
                                                   tag=f"aout{s}", bufs=1,
                                                   name=f"aO{h}_{qcs[s]}")
                                         for s in range(2)]
                                saccs = [atw.tile([128, 512], F32,
                                                  tag=f"sacc{s}", bufs=2,
                                                  name=f"sA{h}_{qcs[s]}")
                                         for s in range(2)]
                                prevs = [None, None]
                                for kp in range(ST // 2):
                                    scps = []
                                    for s in range(2):
                                        scp = psat.tile(
                                            [128, 1024], F32, tag=f"asc{s}",
                                            bufs=1,
                                            name=f"sc{h}_{qcs[s]}_{kp}")
                                        for j in range(2):
                                            k0 = (2 * kp + j) * 128
                                            nc.tensor.matmul(
                                                scp[:, j * 512:(j + 1) * 512],
                                                KTr[h][:, k0:k0 + 128],
                                                QTr[h][:, qsls[s]],
                                                start=True, stop=True)
                                        scps.append(scp)
                                    for s in range(2):
                                        if prevs[s] is not None:
                                            _attn_tail(nc, atw, prevs[s][1],
                                                       prevs[s][0], outps[s],
                                                       saccs[s], V_sb, h,
                                                       prevs[s][0] == 0,
                                                       False, BF16, EXPF, s)
                                        prevs[s] = (kp, scps[s])
                                ocps = []
                                for s in range(2):
                                    _attn_tail(nc, atw, prevs[s][1],
                                               prevs[s][0], outps[s],
                                               saccs[s], V_sb, h,
                                               prevs[s][0] == 0, True,
                                               BF16, EXPF, s)
                                    # eager bf16 PSUM evacuation frees the
                                    # aout slot without waiting for the
                                    # softmax normalization chain
                                    ocp = atw.tile([128, 512], BF16,
                                                   tag=f"ocp{s}", bufs=2,
                                                   name=f"oc{h}_{qcs[s]}")
                                    nc.vector.tensor_copy(ocp[:], outps[s][:])
                                    ocps.append(ocp)
                                if pending is not None:
                                    _flush_round(pending)
                                pending = (h, qsls, saccs, ocps)
                            # AG needs outT complete: flush before its DMA
                            _flush_round(pending)
                            pending = None
                            ao_prev = _wo_load(h - 1) if h > 0 else None
                            nc.sync.dma_start(ag_ins[h][:], outT_sb[h][:])
                            nc.gpsimd.collective_compute(
                                "AllGather", mybir.AluOpType.bypass,
                                replica_groups=GROUPS,
                                ins=[ag_ins[h].opt()],
                                outs=[ag_outs[h].opt()],
                            )
                            if h > 0:
                                _wo_chunk(h - 1, ao_prev)
                        _wo_chunk(HLOC - 1, _wo_load(HLOC - 1))

    nc.compile()
    return nc


def _attn_tail(nc, atw, scp, kp, outp, sacc, V_sb, h, is_first,
               is_last, BF16, EXPF, s=0):
    """exp + out accumulation (PE) + softmax-sum accumulation (DVE)."""
    F32 = sacc.dtype
    ex = atw.tile([128, 1024], BF16, tag=f"exp{s}", name=f"ex{h}_{kp}_{s}")
    nc.scalar.activation(ex[:], scp[:], EXPF, scale=SCALE)
    for j in range(2):
        st = 2 * kp + j
        nc.tensor.matmul(outp[:], V_sb[st][:, h * 128:(h + 1) * 128],
                         ex[:, j * 512:(j + 1) * 512],
                         start=(is_first and j == 0),
                         stop=(is_last and j == 1))
    fold = atw.tile([128, 512], F32, tag=f"fold{s}", bufs=2,
                    name=f"fo{h}_{kp}_{s}")
    nc.vector.tensor_add(fold[:], ex[:, 0:512], ex[:, 512:1024])
    if is_first:
        nc.vector.tensor_copy(sacc[:], fold[:])
    else:
        nc.vector.tensor_add(sacc[:], sacc[:], fold[:])


def _prep_inputs(x, wq, wk, wv, wo, cos, sin):
    """Host-side sharding/layout prep. Returns per-core input dicts."""
    import ml_dtypes
    bf16 = ml_dtypes.bfloat16

    woT = np.ascontiguousarray(wo.T).astype(bf16)         # [E, D]
    cosT = np.ascontiguousarray(cos[:S, :DH].T)           # [DH, S] f32
    sinT = np.ascontiguousarray(sin[:S, :DH].T)
    sinTs = sinT.copy()
    sinTs[:DH // 2] *= -1.0                               # sign-folded rotate
    ones = np.ones((128, 128), np.float32)
    xTs = [np.ascontiguousarray(x[b].T).astype(bf16) for b in range(B)]

    in_maps = []
    for c in range(NCORES):
        hsel = slice((c % 4) * HLOC * DH, ((c % 4) + 1) * HLOC * DH)
        wq_c = wq[hsel, :]                                # [512, D]
        wk_c = wk[hsel, :]
        qk_cols = np.empty((2 * HLOC * DH, D), np.float32)
        for h in range(HLOC):
            qk_cols[(2 * h) * DH:(2 * h + 1) * DH] = \
                wq_c[h * DH:(h + 1) * DH]
            qk_cols[(2 * h + 1) * DH:(2 * h + 2) * DH] = \
                wk_c[h * DH:(h + 1) * DH]
        wqkT = np.ascontiguousarray(qk_cols.T).astype(bf16)   # [D, 1024]
        wvT = np.ascontiguousarray(wv[hsel, :].T).astype(bf16)  # [D, 512]
        in_maps.append({
            "xT": xTs[c // 4],
            "wqkT": wqkT,
            "wvT": wvT,
            "woT": woT,
            "cosT": cosT,
            "sinTs": sinTs,
            "ones": ones,
        })
    return in_maps


def kernel(x, wq, wk, wv, wo, cos, sin):
    global _BUILT
    from concourse.bass_utils import run_bass_kernel_spmd

    if _BUILT is None:
        _BUILT = _build()
    nc = _BUILT

    in_maps = _prep_inputs(
        np.asarray(x, np.float32), np.asarray(wq, np.float32),
        np.asarray(wk, np.float32), np.asarray(wv, np.float32),
        np.asarray(wo, np.float32), np.asarray(cos, np.float32),
        np.asarray(sin, np.float32))

    res = run_bass_kernel_spmd(nc, in_maps, core_ids=list(range(NCORES)))

    out = np.empty((B, S, D), np.float32)
    for c in range(NCORES):
        out[c // 4, (c % 4) * QROWS:((c % 4) + 1) * QROWS, :] = \
            res.results[c]["out"]
    return out


# revision 31
# speedup vs baseline: 1.0659x; 1.0197x over previous
"""Multi-head attention (B=2, S=2048, D=2048, H=16, DH=128, RoPE, non-causal)
on 8 Trainium2 NeuronCores.

Sharding: 2-way data parallel on batch x 4-way tensor parallel on heads.
Core c handles batch c//4 and heads (c%4)*4 .. (c%4)*4+4.

Compute dtype: bf16 on the TensorEngine (fp32 PSUM accumulation, fp32
softmax normalization), which gets FWL weight loads and halves DMA traffic.

Per-core kernel:
  1. QK projections, head-dim-major:  QT/KT[dh, s]  (+ fused RoPE via
     sign-folded sin table and partition-strided rotate-half DMA)
  2. V projection, seq-major:         V[s, dh*4]
  3. Attention, transpose-free: scoresT[k, q] on PE, exp on ACT, out and
     softmax row-sums accumulated on PE (all-ones matmul does the
     partition-axis reduction), normalize with DVE reciprocal+multiply.
  4. Per-head AllGather (groups of 4) of the attention output, overlapped
     with the next head's attention.
  5. Output projection accumulated incrementally per head-group into SBUF,
     for this core's own query slice (cc_rank-based dynamic slicing).
Host gathers the 8 disjoint [512, 2048] fp32 output shards.
"""
import numpy as np

B, S, D, H = 2, 2048, 2048, 16
DH = 128
HLOC = 4                 # heads per core
NCORES = 8
GROUPS = [[0, 1, 2, 3], [4, 5, 6, 7]]
SCALE = 1.0 / np.sqrt(DH)
KT = D // 128            # 16 contraction tiles over the model dim
SC = S // 512            # 4 chunks of 512 along seq
ST = S // 128            # 16 seq tiles of 128
QROWS = S // 4           # 512 output rows per core

_BUILT = None


def _build():
    import concourse.bass as bass
    import concourse.tile as tile
    from concourse import bacc, mybir

    F32 = mybir.dt.float32
    BF16 = mybir.dt.bfloat16
    EXPF = mybir.ActivationFunctionType.Exp

    nc = bacc.Bacc("TRN2", target_bir_lowering=False, debug=False,
                   num_devices=NCORES)

    xT_d = nc.dram_tensor("xT", [D, S], BF16, kind="ExternalInput").ap()
    wqk_d = nc.dram_tensor("wqkT", [D, 2 * HLOC * DH], BF16,
                           kind="ExternalInput").ap()
    wv_d = nc.dram_tensor("wvT", [D, HLOC * DH], BF16,
                          kind="ExternalInput").ap()
    woT_d = nc.dram_tensor("woT", [D, D], BF16, kind="ExternalInput").ap()
    cosT_d = nc.dram_tensor("cosT", [DH, S], F32, kind="ExternalInput").ap()
    sinT_d = nc.dram_tensor("sinTs", [DH, S], F32, kind="ExternalInput").ap()
    ones_d = nc.dram_tensor("ones", [128, 128], F32,
                            kind="ExternalInput").ap()
    out_d = nc.dram_tensor("out", [QROWS, D], F32, kind="ExternalOutput").ap()

    with tile.TileContext(nc) as tc:
        with (
            tc.tile_pool(name="dram", bufs=1, space="DRAM") as dram,
            tc.tile_pool(name="onesp", bufs=1) as onesp,
        ):
            ones_sb = onesp.tile([128, 128], F32)
            nc.sync.dma_start(ones_sb[:], ones_d[:])

            ag_ins = []
            ag_outs = []
            for h in range(HLOC):
                ag_ins.append(dram.tile([DH, S], BF16, name=f"agin{h}"))
                ag_outs.append(dram.tile([4 * DH, S], BF16, name=f"agout{h}"))

            # tiny dummy AllGather to absorb the first-collective warmup
            # cost under the projection phase
            warm_in = dram.tile([128, 128], BF16, name="warmin")
            warm_out = dram.tile([512, 128], BF16, name="warmout")
            nc.sync.dma_start(warm_in[:], ones_d[0:128, 0:64].bitcast(BF16))
            nc.gpsimd.collective_compute(
                "AllGather", mybir.AluOpType.bypass,
                replica_groups=GROUPS,
                ins=[warm_in.opt()], outs=[warm_out.opt()],
            )

            with tc.tile_pool(name="qkr", bufs=1) as qkrp:
                # persistent RoPE'd Q/K, head-dim-major: [dh=128, s=2048] bf16
                QTr = [qkrp.tile([DH, S], BF16, name=f"qtr{h}")
                       for h in range(HLOC)]
                KTr = [qkrp.tile([DH, S], BF16, name=f"ktr{h}")
                       for h in range(HLOC)]

                # ---------------- Phase 1: Q/K projections + RoPE ----------
                with (
                    tc.tile_pool(name="wqk", bufs=1) as wqkp,
                    tc.tile_pool(name="cs", bufs=1) as csp,
                    tc.tile_pool(name="xqk", bufs=12) as xqkp,
                    tc.tile_pool(name="ropew", bufs=6) as ropep,
                    tc.tile_pool(name="psqk", bufs=8, space="PSUM") as psqk,
                ):
                    wqk_sb = [wqkp.tile([128, 2 * HLOC * DH], BF16,
                                        name=f"wqk{kt}") for kt in range(KT)]
                    cos_sb = csp.tile([DH, S], F32)
                    sin_sb = csp.tile([DH, S], F32)
                    # first tiles the PE needs go first in the DMA queues
                    nc.sync.dma_start(wqk_sb[0][:], wqk_d[0:128, :])
                    for sc in range(SC):
                        ssl = slice(sc * 512, (sc + 1) * 512)
                        pss = [psqk.tile([128, 512], F32, tag="qkps",
                                         name=f"qkps{sc}_{t}")
                               for t in range(2 * HLOC)]
                        for kt in range(KT):
                            xc = xqkp.tile([128, 512], BF16, tag="xc",
                                           name=f"xc{sc}_{kt}")
                            nc.sync.dma_start(
                                xc[:], xT_d[kt * 128:(kt + 1) * 128, ssl])
                            if sc == 0 and kt + 1 < KT:
                                nc.sync.dma_start(
                                    wqk_sb[kt + 1][:],
                                    wqk_d[(kt + 1) * 128:(kt + 2) * 128, :])
                            if sc == 0 and kt == KT - 1:
                                nc.sync.dma_start(cos_sb[:], cosT_d[:])
                                nc.sync.dma_start(sin_sb[:], sinT_d[:])
                            for t in range(2 * HLOC):
                                nc.tensor.matmul(
                                    pss[t][:],
                                    wqk_sb[kt][:, t * 128:(t + 1) * 128],
                                    xc[:],
                                    start=(kt == 0), stop=(kt == KT - 1))
                        # RoPE evacuation: qkr = psum*cos + rot(psum)*sin_s
                        # (sc=0 writes are plain copies deferred via cos/sin
                        #  arriving later is fine: deps handle ordering)
                        for t in range(2 * HLOC):
                            h, isk = t // 2, t % 2
                            dst = (KTr[h] if isk else QTr[h])
                            plain = ropep.tile([128, 512], F32, tag="plain",
                                               name=f"pl{sc}_{t}")
                            nc.scalar.copy(plain[:], pss[t][:])
                            tmpc = ropep.tile([128, 512], F32, tag="tmpc",
                                              name=f"tc{sc}_{t}")
                            nc.vector.tensor_mul(tmpc[:], pss[t][:],
                                                 cos_sb[:, ssl])
                            rot = ropep.tile([128, 512], F32, tag="rot",
                                             name=f"ro{sc}_{t}")
                            nc.scalar.dma_start(rot[0:64, :], plain[1::2, :])
                            nc.scalar.dma_start(rot[64:128, :],
                                                plain[0::2, :])
                            rot2 = ropep.tile([128, 512], F32, tag="rot2",
                                              name=f"ro2{sc}_{t}")
                            nc.vector.tensor_mul(rot2[:], rot[:],
                                                 sin_sb[:, ssl])
                            nc.vector.tensor_add(dst[:, ssl], tmpc[:],
                                                 rot2[:])

                with tc.tile_pool(name="vper", bufs=1) as vperp:
                    # persistent V, seq-major: 16 tiles [128 s, 512 = 4 heads]
                    V_sb = [vperp.tile([128, HLOC * DH], BF16,
                                       name=f"v{st}") for st in range(ST)]

                    # ---------------- Phase 2: V projection ----------------
                    with (
                        tc.tile_pool(name="wv", bufs=1) as wvp,
                        tc.tile_pool(name="xv", bufs=8) as xvp,
                        tc.tile_pool(name="psv", bufs=8, space="PSUM") as psv,
                    ):
                        wv_sb = [wvp.tile([128, HLOC * DH], BF16,
                                          name=f"wv{kt}") for kt in range(KT)]
                        nc.sync.dma_start(wv_sb[0][:], wv_d[0:128, :])
                        for half in range(2):
                            pvs = [psv.tile([128, 512], F32, tag="vps",
                                            name=f"vps{half}_{i}")
                                   for i in range(8)]
                            for kt in range(KT):
                                xv = xvp.tile([128, 1024], BF16, tag="xv",
                                              name=f"xv{half}_{kt}")
                                nc.sync.dma_start(
                                    xv[:],
                                    xT_d[kt * 128:(kt + 1) * 128,
                                         half * 1024:(half + 1) * 1024])
                                if half == 0 and kt + 1 < KT:
                                    nc.sync.dma_start(
                                        wv_sb[kt + 1][:],
                                        wv_d[(kt + 1) * 128:(kt + 2) * 128, :])
                                for i in range(8):
                                    nc.tensor.matmul(
                                        pvs[i][:],
                                        xv[:, i * 128:(i + 1) * 128],
                                        wv_sb[kt][:],
                                        start=(kt == 0), stop=(kt == KT - 1))
                            for i in range(8):
                                nc.scalar.copy(V_sb[half * 8 + i][:],
                                               pvs[i][:])

                    # -------- Phase 3+4: attention + AllGather + wo --------
                    with (
                        tc.tile_pool(name="wo", bufs=1) as wop,
                        tc.tile_pool(name="woacc", bufs=1) as woaccp,
                        tc.tile_pool(name="outT", bufs=1) as outTp,
                        tc.tile_pool(name="ao", bufs=2) as aop,
                        tc.tile_pool(name="atw", bufs=3) as atw,
                        tc.tile_pool(name="psat", bufs=1, space="PSUM") as psat,
                    ):
                        rank = nc.sync.cc_rank(replica_groups=GROUPS)
                        # dynamic offsets expressed in f32 units (bf16 pairs):
                        # dynamic ds() on a bf16 AP faults the device
                        qoff32 = rank * 256

                        # woT resident in bf16, loaded during attention
                        wo_sb = [wop.tile([128, D], BF16, name=f"wos{g}")
                                 for g in range(16)]
                        for g in range(16):
                            nc.sync.dma_start(
                                wo_sb[g][:], woT_d[g * 128:(g + 1) * 128, :])

                        wo_acc = [woaccp.tile([128, 512], F32,
                                              name=f"wacc{qt}_{dc}")
                                  for qt in range(4) for dc in range(4)]
                        outT_sb = [outTp.tile([DH, S], BF16, name=f"ot{h}")
                                   for h in range(HLOC)]

                        def _wo_load(h):
                            """Fetch this core's query-slice of AG[h]."""
                            ao_h = [aop.tile([128, 512], BF16, tag=f"aoh{j}",
                                             name=f"aoh{h}_{j}")
                                    for j in range(4)]
                            ag32 = ag_outs[h].bitcast(F32)
                            for j in range(4):
                                nc.sync.dma_start(
                                    ao_h[j][:].bitcast(F32),
                                    ag32[j * 128:(j + 1) * 128,
                                         bass.ds(qoff32, 256)])
                            return ao_h

                        def _wo_chunk(h, ao_h):
                            """Accumulate head-group h's wo contribution."""
                            for qt in range(4):
                                for dc in range(4):
                                    psw = psat.tile([128, 512], F32,
                                                    tag="wops", bufs=2,
                                                    name=f"wps{h}_{qt}_{dc}")
                                    for j in range(4):
                                        nc.tensor.matmul(
                                            psw[:],
                                            ao_h[j][:,
                                                    qt * 128:(qt + 1) * 128],
                                            wo_sb[4 * j + h][:,
                                                             dc * 512:
                                                             (dc + 1) * 512],
                                            start=(j == 0), stop=(j == 3))
                                    acc = wo_acc[qt * 4 + dc]
                                    if h == 0:
                                        nc.scalar.copy(acc[:], psw[:])
                                    elif h == HLOC - 1:
                                        nc.vector.tensor_add(acc[:], acc[:],
                                                             psw[:])
                                        nc.sync.dma_start(
                                            out_d[qt * 128:(qt + 1) * 128,
                                                  dc * 512:(dc + 1) * 512],
                                            acc[:])
                                    else:
                                        nc.vector.tensor_add(acc[:], acc[:],
                                                             psw[:])

                        def _flush_round(p):
                            """softmax-sum matmul + normalization for a
                            finished round, deferred so it overlaps the next
                            round's PE work instead of blocking it."""
                            ph, pqsls, psaccs, pocps = p
                            for s in range(2):
                                sump = psat.tile([128, 512], F32,
                                                 tag="wops", bufs=2,
                                                 name=f"aS{ph}_{pqsls[s].start}")
                                nc.tensor.matmul(sump[:], ones_sb[:],
                                                 psaccs[s][:],
                                                 start=True, stop=True)
                                rec = atw.tile([128, 512], F32, tag="rec",
                                               name=f"rc{ph}_{pqsls[s].start}")
                                nc.vector.reciprocal(rec[:], sump[:])
                                nc.vector.tensor_mul(
                                    outT_sb[ph][:, pqsls[s]],
                                    pocps[s][:].bitcast(F32)
                                    if False else pocps[s][:],
                                    rec[:])

                        pending = None
                        for h in range(HLOC):
                            # two interleaved query-chunk streams: while one
                            # stream waits on exp (ACT), the other keeps the
                            # PE busy
                            for qp in range(SC // 2):
                                qcs = (2 * qp, 2 * qp + 1)
                                qsls = [slice(qc * 512, (qc + 1) * 512)
                                        for qc in qcs]
                                outps = [psat.tile([128, 512], F32,
                                                   tag=f"aout{s}", bufs=1,
                                                   name=f"aO{h}_{qcs[s]}")
                                         for s in range(2)]
                                saccs = [atw.tile([128, 512], F32,
                                                  tag=f"sacc{s}", bufs=2,
                                                  name=f"sA{h}_{qcs[s]}")
                                         for s in range(2)]
                                prevs = [None, None]
                                for kp in range(ST // 2):
                                    scps = []
                                    for s in range(2):
                                        scp = psat.tile(
                                            [128, 1024], F32, tag=f"asc{s}",
                                            bufs=1,
                                            name=f"sc{h}_{qcs[s]}_{kp}")
                                        for j in range(2):
                                            k0 = (2 * kp + j) * 128
                                            nc.tensor.matmul(
                                                scp[:, j * 512:(j + 1) * 512],
                                                KTr[h][:, k0:k0 + 128],
                                                QTr[h][:, qsls[s]],
                                                start=True, stop=True)
                                        scps.append(scp)
                                    for s in range(2):
                                        if prevs[s] is not None:
                                            _attn_tail(nc, atw, prevs[s][1],
                                                       prevs[s][0], outps[s],
                                                       saccs[s], V_sb, h,
                                                       prevs[s][0] == 0,
                                                       False, BF16, EXPF, s)
                                        prevs[s] = (kp, scps[s])
                                ocps = []
                                for s in range(2):
                                    _attn_tail(nc, atw, prevs[s][1],
                                               prevs[s][0], outps[s],
                                               saccs[s], V_sb, h,
                                               prevs[s][0] == 0, True,
                                               BF16, EXPF, s)
                                    # eager bf16 PSUM evacuation frees the
                                    # aout slot without waiting for the
                                    # softmax normalization chain
                                    ocp = atw.tile([128, 512], BF16,
                                                   tag=f"ocp{s}", bufs=2,
                                                   name=f"oc{h}_{qcs[s]}")
                                    nc.vector.tensor_copy(ocp[:], outps[s][:])
                                    ocps.append(ocp)
                                if pending is not None:
                                    _flush_round(pending)
                                pending = (h, qsls, saccs, ocps)
                            # AG needs outT complete: flush before its DMA
                            _flush_round(pending)
                            pending = None
                            ao_prev = _wo_load(h - 1) if h > 0 else None
                            nc.sync.dma_start(ag_ins[h][:], outT_sb[h][:])
                            nc.gpsimd.collective_compute(
                                "AllGather", mybir.AluOpType.bypass,
                                replica_groups=GROUPS,
                                ins=[ag_ins[h].opt()],
                                outs=[ag_outs[h].opt()],
                            )
                            if h > 0:
                                _wo_chunk(h - 1, ao_prev)
                        _wo_chunk(HLOC - 1, _wo_load(HLOC - 1))

    nc.compile()
    return nc


def _attn_tail(nc, atw, scp, kp, outp, sacc, V_sb, h, is_first,
               is_last, BF16, EXPF, s=0):
    """exp + out accumulation (PE) + softmax-sum accumulation (DVE)."""
    F32 = sacc.dtype
    ex = atw.tile([128, 1024], BF16, tag=f"exp{s}", name=f"ex{h}_{kp}_{s}")
    nc.scalar.activation(ex[:], scp[:], EXPF, scale=SCALE)
    for j in range(2):
        st = 2 * kp + j
        nc.tensor.matmul(outp[:], V_sb[st][:, h * 128:(h + 1) * 128],
                         ex[:, j * 512:(j + 1) * 512],
                         start=(is_first and j == 0),
                         stop=(is_last and j == 1))
    fold = atw.tile([128, 512], F32, tag=f"fold{s}", bufs=2,
                    name=f"fo{h}_{kp}_{s}")
    nc.vector.tensor_add(fold[:], ex[:, 0:512], ex[:, 512:1024])
    if is_first:
        nc.vector.tensor_copy(sacc[:], fold[:])
    else:
        nc.vector.tensor_add(sacc[:], sacc[:], fold[:])


def _prep_inputs(x, wq, wk, wv, wo, cos, sin):
    """Host-side sharding/layout prep. Returns per-core input dicts."""
    import ml_dtypes
    bf16 = ml_dtypes.bfloat16

    woT = np.ascontiguousarray(wo.T).astype(bf16)         # [E, D]
    cosT = np.ascontiguousarray(cos[:S, :DH].T)           # [DH, S] f32
    sinT = np.ascontiguousarray(sin[:S, :DH].T)
    sinTs = sinT.copy()
    sinTs[:DH // 2] *= -1.0                               # sign-folded rotate
    ones = np.ones((128, 128), np.float32)
    xTs = [np.ascontiguousarray(x[b].T).astype(bf16) for b in range(B)]

    in_maps = []
    for c in range(NCORES):
        hsel = slice((c % 4) * HLOC * DH, ((c % 4) + 1) * HLOC * DH)
        wq_c = wq[hsel, :]                                # [512, D]
        wk_c = wk[hsel, :]
        qk_cols = np.empty((2 * HLOC * DH, D), np.float32)
        for h in range(HLOC):
            qk_cols[(2 * h) * DH:(2 * h + 1) * DH] = \
                wq_c[h * DH:(h + 1) * DH]
            qk_cols[(2 * h + 1) * DH:(2 * h + 2) * DH] = \
                wk_c[h * DH:(h + 1) * DH]
        wqkT = np.ascontiguousarray(qk_cols.T).astype(bf16)   # [D, 1024]
        wvT = np.ascontiguousarray(wv[hsel, :].T).astype(bf16)  # [D, 512]
        in_maps.append({
            "xT": xTs[c // 4],
            "wqkT": wqkT,
            "wvT": wvT,
            "woT": woT,
            "cosT": cosT,
            "sinTs": sinTs,
            "ones": ones,
        })
    return in_maps


def kernel(x, wq, wk, wv, wo, cos, sin):
    global _BUILT
    from concourse.bass_utils import run_bass_kernel_spmd

    if _BUILT is None:
        _BUILT = _build()
    nc = _BUILT

    in_maps = _prep_inputs(
        np.asarray(x, np.float32), np.asarray(wq, np.float32),
        np.asarray(wk, np.float32), np.asarray(wv, np.float32),
        np.asarray(wo, np.float32), np.asarray(cos, np.float32),
        np.asarray(sin, np.float32))

    res = run_bass_kernel_spmd(nc, in_maps, core_ids=list(range(NCORES)))

    out = np.empty((B, S, D), np.float32)
    for c in range(NCORES):
        out[c // 4, (c % 4) * QROWS:((c % 4) + 1) * QROWS, :] = \
            res.results[c]["out"]
    return out


# revision 32
# speedup vs baseline: 1.0814x; 1.0145x over previous
"""Multi-head attention (B=2, S=2048, D=2048, H=16, DH=128, RoPE, non-causal)
on 8 Trainium2 NeuronCores.

Sharding: 2-way data parallel on batch x 4-way tensor parallel on heads.
Core c handles batch c//4 and heads (c%4)*4 .. (c%4)*4+4.

Compute dtype: bf16 on the TensorEngine (fp32 PSUM accumulation, fp32
softmax normalization), which gets FWL weight loads and halves DMA traffic.

Per-core kernel:
  1. QK projections, head-dim-major:  QT/KT[dh, s]  (+ fused RoPE via
     sign-folded sin table and partition-strided rotate-half DMA)
  2. V projection, seq-major:         V[s, dh*4]
  3. Attention, transpose-free: scoresT[k, q] on PE, exp on ACT, out and
     softmax row-sums accumulated on PE (all-ones matmul does the
     partition-axis reduction), normalize with DVE reciprocal+multiply.
  4. Per-head AllGather (groups of 4) of the attention output, overlapped
     with the next head's attention.
  5. Output projection accumulated incrementally per head-group into SBUF,
     for this core's own query slice (cc_rank-based dynamic slicing).
Host gathers the 8 disjoint [512, 2048] fp32 output shards.
"""
import numpy as np

B, S, D, H = 2, 2048, 2048, 16
DH = 128
HLOC = 4                 # heads per core
NCORES = 8
GROUPS = [[0, 1, 2, 3], [4, 5, 6, 7]]
SCALE = 1.0 / np.sqrt(DH)
KT = D // 128            # 16 contraction tiles over the model dim
SC = S // 512            # 4 chunks of 512 along seq
ST = S // 128            # 16 seq tiles of 128
QROWS = S // 4           # 512 output rows per core

_BUILT = None


def _build():
    import concourse.bass as bass
    import concourse.tile as tile
    from concourse import bacc, mybir

    F32 = mybir.dt.float32
    BF16 = mybir.dt.bfloat16
    EXPF = mybir.ActivationFunctionType.Exp

    nc = bacc.Bacc("TRN2", target_bir_lowering=False, debug=False,
                   num_devices=NCORES)

    xT_d = nc.dram_tensor("xT", [D, S], BF16, kind="ExternalInput").ap()
    wqk_d = nc.dram_tensor("wqkT", [D, 2 * HLOC * DH], BF16,
                           kind="ExternalInput").ap()
    wv_d = nc.dram_tensor("wvT", [D, HLOC * DH], BF16,
                          kind="ExternalInput").ap()
    woT_d = nc.dram_tensor("woT", [D, D], BF16, kind="ExternalInput").ap()
    cosT_d = nc.dram_tensor("cosT", [DH, S], F32, kind="ExternalInput").ap()
    sinT_d = nc.dram_tensor("sinTs", [DH, S], F32, kind="ExternalInput").ap()
    ones_d = nc.dram_tensor("ones", [128, 128], F32,
                            kind="ExternalInput").ap()
    out_d = nc.dram_tensor("out", [QROWS, D], F32, kind="ExternalOutput").ap()

    with tile.TileContext(nc) as tc:
        with (
            tc.tile_pool(name="dram", bufs=1, space="DRAM") as dram,
            tc.tile_pool(name="onesp", bufs=1) as onesp,
        ):
            F32R = mybir.dt.float32r
            ones_sb = onesp.tile([128, 128], F32R)
            nc.sync.dma_start(ones_sb[:], ones_d[:].bitcast(F32R))

            ag_ins = []
            ag_outs = []
            for h in range(HLOC):
                ag_ins.append(dram.tile([DH, S], BF16, name=f"agin{h}"))
                ag_outs.append(dram.tile([4 * DH, S], BF16, name=f"agout{h}"))

            # tiny dummy AllGather to absorb the first-collective warmup
            # cost under the projection phase
            warm_in = dram.tile([128, 128], BF16, name="warmin")
            warm_out = dram.tile([512, 128], BF16, name="warmout")
            nc.sync.dma_start(warm_in[:], ones_d[0:128, 0:64].bitcast(BF16))
            nc.gpsimd.collective_compute(
                "AllGather", mybir.AluOpType.bypass,
                replica_groups=GROUPS,
                ins=[warm_in.opt()], outs=[warm_out.opt()],
            )

            with tc.tile_pool(name="qkr", bufs=1) as qkrp:
                # persistent RoPE'd Q/K, head-dim-major: [dh=128, s=2048] bf16
                QTr = [qkrp.tile([DH, S], BF16, name=f"qtr{h}")
                       for h in range(HLOC)]
                KTr = [qkrp.tile([DH, S], BF16, name=f"ktr{h}")
                       for h in range(HLOC)]

                # ---------------- Phase 1: Q/K projections + RoPE ----------
                with (
                    tc.tile_pool(name="wqk", bufs=1) as wqkp,
                    tc.tile_pool(name="cs", bufs=1) as csp,
                    tc.tile_pool(name="xqk", bufs=12) as xqkp,
                    tc.tile_pool(name="ropew", bufs=6) as ropep,
                    tc.tile_pool(name="psqk", bufs=8, space="PSUM") as psqk,
                ):
                    wqk_sb = [wqkp.tile([128, 2 * HLOC * DH], BF16,
                                        name=f"wqk{kt}") for kt in range(KT)]
                    cos_sb = csp.tile([DH, S], F32)
                    sin_sb = csp.tile([DH, S], F32)
                    # first tiles the PE needs go first in the DMA queues
                    nc.sync.dma_start(wqk_sb[0][:], wqk_d[0:128, :])
                    for sc in range(SC):
                        ssl = slice(sc * 512, (sc + 1) * 512)
                        pss = [psqk.tile([128, 512], F32, tag="qkps",
                                         name=f"qkps{sc}_{t}")
                               for t in range(2 * HLOC)]
                        for kt in range(KT):
                            xc = xqkp.tile([128, 512], BF16, tag="xc",
                                           name=f"xc{sc}_{kt}")
                            nc.sync.dma_start(
                                xc[:], xT_d[kt * 128:(kt + 1) * 128, ssl])
                            if sc == 0 and kt + 1 < KT:
                                nc.sync.dma_start(
                                    wqk_sb[kt + 1][:],
                                    wqk_d[(kt + 1) * 128:(kt + 2) * 128, :])
                            if sc == 0 and kt == KT - 1:
                                nc.sync.dma_start(cos_sb[:], cosT_d[:])
                                nc.sync.dma_start(sin_sb[:], sinT_d[:])
                            for t in range(2 * HLOC):
                                nc.tensor.matmul(
                                    pss[t][:],
                                    wqk_sb[kt][:, t * 128:(t + 1) * 128],
                                    xc[:],
                                    start=(kt == 0), stop=(kt == KT - 1))
                        # RoPE evacuation: qkr = psum*cos + rot(psum)*sin_s
                        # (sc=0 writes are plain copies deferred via cos/sin
                        #  arriving later is fine: deps handle ordering)
                        for t in range(2 * HLOC):
                            h, isk = t // 2, t % 2
                            dst = (KTr[h] if isk else QTr[h])
                            plain = ropep.tile([128, 512], F32, tag="plain",
                                               name=f"pl{sc}_{t}")
                            nc.scalar.copy(plain[:], pss[t][:])
                            tmpc = ropep.tile([128, 512], F32, tag="tmpc",
                                              name=f"tc{sc}_{t}")
                            nc.vector.tensor_mul(tmpc[:], pss[t][:],
                                                 cos_sb[:, ssl])
                            rot = ropep.tile([128, 512], F32, tag="rot",
                                             name=f"ro{sc}_{t}")
                            nc.scalar.dma_start(rot[0:64, :], plain[1::2, :])
                            nc.scalar.dma_start(rot[64:128, :],
                                                plain[0::2, :])
                            rot2 = ropep.tile([128, 512], F32, tag="rot2",
                                              name=f"ro2{sc}_{t}")
                            nc.vector.tensor_mul(rot2[:], rot[:],
                                                 sin_sb[:, ssl])
                            nc.vector.tensor_add(dst[:, ssl], tmpc[:],
                                                 rot2[:])

                with tc.tile_pool(name="vper", bufs=1) as vperp:
                    # persistent V, seq-major: 16 tiles [128 s, 512 = 4 heads]
                    V_sb = [vperp.tile([128, HLOC * DH], BF16,
                                       name=f"v{st}") for st in range(ST)]

                    # ---------------- Phase 2: V projection ----------------
                    with (
                        tc.tile_pool(name="wv", bufs=1) as wvp,
                        tc.tile_pool(name="xv", bufs=8) as xvp,
                        tc.tile_pool(name="psv", bufs=8, space="PSUM") as psv,
                    ):
                        wv_sb = [wvp.tile([128, HLOC * DH], BF16,
                                          name=f"wv{kt}") for kt in range(KT)]
                        nc.sync.dma_start(wv_sb[0][:], wv_d[0:128, :])
                        for half in range(2):
                            pvs = [psv.tile([128, 512], F32, tag="vps",
                                            name=f"vps{half}_{i}")
                                   for i in range(8)]
                            for kt in range(KT):
                                xv = xvp.tile([128, 1024], BF16, tag="xv",
                                              name=f"xv{half}_{kt}")
                                nc.sync.dma_start(
                                    xv[:],
                                    xT_d[kt * 128:(kt + 1) * 128,
                                         half * 1024:(half + 1) * 1024])
                                if half == 0 and kt + 1 < KT:
                                    nc.sync.dma_start(
                                        wv_sb[kt + 1][:],
                                        wv_d[(kt + 1) * 128:(kt + 2) * 128, :])
                                for i in range(8):
                                    nc.tensor.matmul(
                                        pvs[i][:],
                                        xv[:, i * 128:(i + 1) * 128],
                                        wv_sb[kt][:],
                                        start=(kt == 0), stop=(kt == KT - 1))
                            for i in range(8):
                                nc.scalar.copy(V_sb[half * 8 + i][:],
                                               pvs[i][:])

                    # -------- Phase 3+4: attention + AllGather + wo --------
                    with (
                        tc.tile_pool(name="wo", bufs=1) as wop,
                        tc.tile_pool(name="woacc", bufs=1) as woaccp,
                        tc.tile_pool(name="outT", bufs=1) as outTp,
                        tc.tile_pool(name="ao", bufs=2) as aop,
                        tc.tile_pool(name="atw", bufs=3) as atw,
                        tc.tile_pool(name="psat", bufs=1, space="PSUM") as psat,
                    ):
                        rank = nc.sync.cc_rank(replica_groups=GROUPS)
                        # dynamic offsets expressed in f32 units (bf16 pairs):
                        # dynamic ds() on a bf16 AP faults the device
                        qoff32 = rank * 256

                        # woT resident in bf16, loaded during attention
                        wo_sb = [wop.tile([128, D], BF16, name=f"wos{g}")
                                 for g in range(16)]
                        for g in range(16):
                            nc.sync.dma_start(
                                wo_sb[g][:], woT_d[g * 128:(g + 1) * 128, :])

                        wo_acc = [woaccp.tile([128, 512], F32,
                                              name=f"wacc{qt}_{dc}")
                                  for qt in range(4) for dc in range(4)]
                        outT_sb = [outTp.tile([DH, S], BF16, name=f"ot{h}")
                                   for h in range(HLOC)]

                        def _wo_load(h):
                            """Fetch this core's query-slice of AG[h]."""
                            ao_h = [aop.tile([128, 512], BF16, tag=f"aoh{j}",
                                             name=f"aoh{h}_{j}")
                                    for j in range(4)]
                            ag32 = ag_outs[h].bitcast(F32)
                            for j in range(4):
                                nc.sync.dma_start(
                                    ao_h[j][:].bitcast(F32),
                                    ag32[j * 128:(j + 1) * 128,
                                         bass.ds(qoff32, 256)])
                            return ao_h

                        def _wo_chunk(h, ao_h):
                            """Accumulate head-group h's wo contribution."""
                            for qt in range(4):
                                for dc in range(4):
                                    psw = psat.tile([128, 512], F32,
                                                    tag="wops", bufs=2,
                                                    name=f"wps{h}_{qt}_{dc}")
                                    for j in range(4):
                                        nc.tensor.matmul(
                                            psw[:],
                                            ao_h[j][:,
                                                    qt * 128:(qt + 1) * 128],
                                            wo_sb[4 * j + h][:,
                                                             dc * 512:
                                                             (dc + 1) * 512],
                                            start=(j == 0), stop=(j == 3))
                                    acc = wo_acc[qt * 4 + dc]
                                    if h == 0:
                                        nc.scalar.copy(acc[:], psw[:])
                                    elif h == HLOC - 1:
                                        nc.vector.tensor_add(acc[:], acc[:],
                                                             psw[:])
                                        nc.sync.dma_start(
                                            out_d[qt * 128:(qt + 1) * 128,
                                                  dc * 512:(dc + 1) * 512],
                                            acc[:])
                                    else:
                                        nc.vector.tensor_add(acc[:], acc[:],
                                                             psw[:])

                        def _flush_round(p):
                            """softmax-sum matmul + normalization for a
                            finished round, deferred so it overlaps the next
                            round's PE work instead of blocking it."""
                            ph, pqsls, psaccs, pocps = p
                            for s in range(2):
                                sump = psat.tile([128, 512], F32,
                                                 tag="wops", bufs=2,
                                                 name=f"aS{ph}_{pqsls[s].start}")
                                nc.tensor.matmul(sump[:], ones_sb[:],
                                                 psaccs[s][:],
                                                 start=True, stop=True)
                                rec = atw.tile([128, 512], F32, tag="rec",
                                               name=f"rc{ph}_{pqsls[s].start}")
                                nc.vector.reciprocal(rec[:], sump[:])
                                nc.vector.tensor_mul(
                                    outT_sb[ph][:, pqsls[s]],
                                    pocps[s][:].bitcast(F32)
                                    if False else pocps[s][:],
                                    rec[:])

                        pending = None
                        for h in range(HLOC):
                            # two interleaved query-chunk streams: while one
                            # stream waits on exp (ACT), the other keeps the
                            # PE busy
                            for qp in range(SC // 2):
                                qcs = (2 * qp, 2 * qp + 1)
                                qsls = [slice(qc * 512, (qc + 1) * 512)
                                        for qc in qcs]
                                outps = [psat.tile([128, 512], F32,
                                                   tag=f"aout{s}", bufs=1,
                                                   name=f"aO{h}_{qcs[s]}")
                                         for s in range(2)]
                                saccs = [atw.tile([128, 512], F32R,
                                                  tag=f"sacc{s}", bufs=2,
                                                  name=f"sA{h}_{qcs[s]}")
                                         for s in range(2)]
                                prevs = [None, None]
                                for kp in range(ST // 2):
                                    scps = []
                                    for s in range(2):
                                        scp = psat.tile(
                                            [128, 1024], F32, tag=f"asc{s}",
                                            bufs=1,
                                            name=f"sc{h}_{qcs[s]}_{kp}")
                                        for j in range(2):
                                            k0 = (2 * kp + j) * 128
                                            nc.tensor.matmul(
                                                scp[:, j * 512:(j + 1) * 512],
                                                KTr[h][:, k0:k0 + 128],
                                                QTr[h][:, qsls[s]],
                                                start=True, stop=True)
                                        scps.append(scp)
                                    for s in range(2):
                                        if prevs[s] is not None:
                                            _attn_tail(nc, atw, prevs[s][1],
                                                       prevs[s][0], outps[s],
                                                       saccs[s], V_sb, h,
                                                       prevs[s][0] == 0,
                                                       False, BF16, EXPF, s)
                                        prevs[s] = (kp, scps[s])
                                ocps = []
                                for s in range(2):
                                    _attn_tail(nc, atw, prevs[s][1],
                                               prevs[s][0], outps[s],
                                               saccs[s], V_sb, h,
                                               prevs[s][0] == 0, True,
                                               BF16, EXPF, s)
                                    # eager bf16 PSUM evacuation frees the
                                    # aout slot without waiting for the
                                    # softmax normalization chain
                                    ocp = atw.tile([128, 512], BF16,
                                                   tag=f"ocp{s}", bufs=2,
                                                   name=f"oc{h}_{qcs[s]}")
                                    nc.vector.tensor_copy(ocp[:], outps[s][:])
                                    ocps.append(ocp)
                                if pending is not None:
                                    _flush_round(pending)
                                pending = (h, qsls, saccs, ocps)
                            # AG needs outT complete: flush before its DMA
                            _flush_round(pending)
                            pending = None
                            ao_prev = _wo_load(h - 1) if h > 0 else None
                            nc.sync.dma_start(ag_ins[h][:], outT_sb[h][:])
                            nc.gpsimd.collective_compute(
                                "AllGather", mybir.AluOpType.bypass,
                                replica_groups=GROUPS,
                                ins=[ag_ins[h].opt()],
                                outs=[ag_outs[h].opt()],
                            )
                            if h > 0:
                                _wo_chunk(h - 1, ao_prev)
                        _wo_chunk(HLOC - 1, _wo_load(HLOC - 1))

    nc.compile()
    return nc


def _attn_tail(nc, atw, scp, kp, outp, sacc, V_sb, h, is_first,
               is_last, BF16, EXPF, s=0):
    """exp + out accumulation (PE) + softmax-sum accumulation (DVE)."""
    F32 = sacc.dtype
    ex = atw.tile([128, 1024], BF16, tag=f"exp{s}", name=f"ex{h}_{kp}_{s}")
    nc.scalar.activation(ex[:], scp[:], EXPF, scale=SCALE)
    for j in range(2):
        st = 2 * kp + j
        nc.tensor.matmul(outp[:], V_sb[st][:, h * 128:(h + 1) * 128],
                         ex[:, j * 512:(j + 1) * 512],
                         start=(is_first and j == 0),
                         stop=(is_last and j == 1))
    fold = atw.tile([128, 512], F32, tag=f"fold{s}", bufs=2,
                    name=f"fo{h}_{kp}_{s}")
    nc.vector.tensor_add(fold[:], ex[:, 0:512], ex[:, 512:1024])
    if is_first:
        nc.vector.tensor_copy(sacc[:], fold[:])
    else:
        nc.vector.tensor_add(sacc[:], sacc[:].bitcast(fold.dtype), fold[:])


def _prep_inputs(x, wq, wk, wv, wo, cos, sin):
    """Host-side sharding/layout prep. Returns per-core input dicts."""
    import ml_dtypes
    bf16 = ml_dtypes.bfloat16

    woT = np.ascontiguousarray(wo.T).astype(bf16)         # [E, D]
    cosT = np.ascontiguousarray(cos[:S, :DH].T)           # [DH, S] f32
    sinT = np.ascontiguousarray(sin[:S, :DH].T)
    sinTs = sinT.copy()
    sinTs[:DH // 2] *= -1.0                               # sign-folded rotate
    ones = np.ones((128, 128), np.float32)
    xTs = [np.ascontiguousarray(x[b].T).astype(bf16) for b in range(B)]

    in_maps = []
    for c in range(NCORES):
        hsel = slice((c % 4) * HLOC * DH, ((c % 4) + 1) * HLOC * DH)
        wq_c = wq[hsel, :]                                # [512, D]
        wk_c = wk[hsel, :]
        qk_cols = np.empty((2 * HLOC * DH, D), np.float32)
        for h in range(HLOC):
            qk_cols[(2 * h) * DH:(2 * h + 1) * DH] = \
                wq_c[h * DH:(h + 1) * DH]
            qk_cols[(2 * h + 1) * DH:(2 * h + 2) * DH] = \
                wk_c[h * DH:(h + 1) * DH]
        wqkT = np.ascontiguousarray(qk_cols.T).astype(bf16)   # [D, 1024]
        wvT = np.ascontiguousarray(wv[hsel, :].T).astype(bf16)  # [D, 512]
        in_maps.append({
            "xT": xTs[c // 4],
            "wqkT": wqkT,
            "wvT": wvT,
            "woT": woT,
            "cosT": cosT,
            "sinTs": sinTs,
            "ones": ones,
        })
    return in_maps


def kernel(x, wq, wk, wv, wo, cos, sin):
    global _BUILT
    from concourse.bass_utils import run_bass_kernel_spmd

    if _BUILT is None:
        _BUILT = _build()
    nc = _BUILT

    in_maps = _prep_inputs(
        np.asarray(x, np.float32), np.asarray(wq, np.float32),
        np.asarray(wk, np.float32), np.asarray(wv, np.float32),
        np.asarray(wo, np.float32), np.asarray(cos, np.float32),
        np.asarray(sin, np.float32))

    res = run_bass_kernel_spmd(nc, in_maps, core_ids=list(range(NCORES)))

    out = np.empty((B, S, D), np.float32)
    for c in range(NCORES):
        out[c // 4, (c % 4) * QROWS:((c % 4) + 1) * QROWS, :] = \
            res.results[c]["out"]
    return out
